# revision 28
# baseline (speedup 1.0000x reference)
"""Trainium2 Bass kernel for nn_AngularDescriptor (gnn_message_passing).

Legendre addition theorem factorization:
  q[i,d,l] = 0.5 * ( sum_{m in shell l} A[i,d,m]^2  -  B[i,d] )
  A[i,d,m] = sum_j g_ij[d] * Yhat_m(u_ij),   B[i,d] = sum_j g_ij[d]^2

v4: pairs-on-partitions layout.  Partition p = j*6 + a (j = neighbor
0..19, a = atom lane 0..5), free dim = 210 atom-chunks per core.  All
index gathers are host-marshaled dense slabs.  The A and B contractions
over j run on the tensor engine: per chunk a block-diagonal weight
matrix W[(j,a),(a',d)] = g * (a==a') multiplies Y -> PSUM A[(a,d), m];
B uses a fixed block-ones lhsT.  The 0.5*q - 0.5*B combine and the
output unscramble happen on host (pure reshapes + one subtract).
"""
import os
import sys

sys.path.insert(0, "/opt/trn_rl_repo")
os.environ.setdefault("NEURON_RT_RESET_CORES", "1")

import math
import numpy as np

from concourse import bacc, bass, mybir, tile
from concourse.bass_utils import run_bass_kernel_spmd

N_ATOMS = 10000
M_NBR = 20
N_TYPES = 4
N_DESC = 8
K_MAX = 8
L_MAX = 4
R_C = 5.0

NCORES = 8
A6 = 6                      # atom lanes per partition group
PP = M_NBR * A6             # 120 active partitions
CH = 210                    # atom chunks per core
HB = 105                    # chunks per half (software pipeline)
CGRP = 21                   # chunks per PSUM group (21*16 = 336 cols)
CA = A6 * CH                # atoms per core = 1260
NTOT = NCORES * CA          # padded atom count = 10080
F32 = mybir.dt.float32
BF16 = mybir.dt.bfloat16

SQ3 = math.sqrt(3.0)
C31 = math.sqrt(3.0 / 8.0)
C32 = math.sqrt(15.0)
C33 = math.sqrt(5.0 / 8.0)
SHELL_OFF = [0, 1, 4, 9, 16]
SQH = math.sqrt(0.5)


def _ap(t, off, dims, parts=PP):
    """Custom free-dim AP on a tile: dims = [(step, count), ...]."""
    base = t[:]
    ap = [[base.ap[0][0], parts]] + [[s, c] for (s, c) in dims]
    return bass.AP(base.tensor, base.offset + off, ap)


def build_nc(debug=False):
    nc = bacc.Bacc()
    posj_d = nc.declare_dram_parameter("posj", [128, CH * 3], F32,
                                       isOutput=False)
    ctr_d = nc.declare_dram_parameter("ctr", [128, CH * 3], F32,
                                      isOutput=False)
    cpair_d = nc.declare_dram_parameter("cpair", [128, CH * 64], BF16,
                                        isOutput=False)
    mk_d = nc.declare_dram_parameter("mk", [128, 48], BF16, isOutput=False)
    wones_d = nc.declare_dram_parameter("wones", [128, A6], BF16,
                                        isOutput=False)
    qa_d = nc.declare_dram_parameter("qa", [48, CH * L_MAX], F32,
                                     isOutput=True)
    braw_d = nc.declare_dram_parameter("braw", [A6, 2 * N_DESC * HB], F32,
                                       isOutput=True)

    with tile.TileContext(nc) as tc:
        with tc.tile_pool(name="main", bufs=1) as pool, \
             tc.tile_pool(name="psA", bufs=3, space="PSUM") as psA, \
             tc.tile_pool(name="psB", bufs=2, space="PSUM") as psB:
            consts = pool.tile([128, 2], F32)
            nc.vector.memset(consts[:, 0:1], math.pi / 2)
            nc.const_aps.aps[(F32, math.pi / 2)] = consts[:, 0:1]
            nc.vector.memset(consts[:, 1:2], -math.sqrt(2.0))
            nc.const_aps.aps[(F32, -math.sqrt(2.0))] = consts[:, 1:2]

            posj = pool.tile([128, CH * 3], F32)
            ctr = pool.tile([128, CH * 3], F32)
            cpair = pool.tile([128, CH * 64], BF16)   # becomes gtmp/t1
            mk = pool.tile([128, 48], BF16)
            wones = pool.tile([128, A6], BF16)
            gT = pool.tile([128, CH * N_DESC], BF16)
            nc.sync.dma_start(out=posj[:], in_=posj_d[:])
            nc.sync.dma_start(out=ctr[:], in_=ctr_d[:])
            nc.sync.dma_start(out=wones[:], in_=wones_d[:])
            nc.sync.dma_start(out=mk[:], in_=mk_d[:])
            nc.sync.dma_start(out=cpair[:], in_=cpair_d[:])

            dxyz = pool.tile([128, CH * 3], F32)
            sqt = pool.tile([128, CH * 3], F32)
            r2 = pool.tile([128, CH], F32)
            rinv = pool.tile([128, CH], F32)
            u = pool.tile([128, CH * 3], F32)
            s01 = pool.tile([128, CH], F32)
            cosx = pool.tile([128, CH], F32)
            mask = pool.tile([128, CH], F32)
            fch = pool.tile([128, CH], F32)
            tmp0 = pool.tile([128, CH], F32)
            tm1 = pool.tile([128, CH], F32)
            xc = pool.tile([128, CH], F32)
            x2 = pool.tile([128, CH], F32)
            Tall = pool.tile([128, CH * K_MAX], F32)
            f = pool.tile([128, CH * K_MAX], BF16)
            g = pool.tile([128, CH * N_DESC], BF16)
            gsq = pool.tile([128, CH * N_DESC], BF16)
            W = pool.tile([128, CH * 48], BF16)
            Y = pool.tile([128, CH * 16], F32)
            Y16 = pool.tile([128, CH * 16], BF16)
            x2c = pool.tile([128, CH], F32)
            y2c = pool.tile([128, CH], F32)
            z2c = pool.tile([128, CH], F32)
            xyc = pool.tile([128, CH], F32)
            dxyc = pool.tile([128, CH], F32)
            tl3 = pool.tile([128, CH], F32)
            tl4 = pool.tile([128, CH], F32)
            tl5 = pool.tile([128, CH], F32)
            tl6 = pool.tile([128, CH], F32)
            uzs = pool.tile([128, CH], F32)
            uzC = pool.tile([128, CH], F32)
            dxyh = pool.tile([128, CH], F32)
            y3c = pool.tile([128, CH], F32)
            x3c = pool.tile([128, CH], F32)
            uxC = pool.tile([128, CH], F32)
            uyC = pool.tile([128, CH], F32)
            Araw = pool.tile([128, CH * 16], F32)
            qa = pool.tile([128, CH * L_MAX], F32)
            braw = pool.tile([128, 2 * N_DESC * HB], F32)

            TT = mybir.AluOpType
            AF = mybir.ActivationFunctionType

            # Y16[m=0] = sqrt(0.5); slots m>=1 overwritten by the convert
            nc.gpsimd.memset(Y16[0:PP, :], SQH)

            def fl(t, w):
                return _ap(t, 0, [(1, CH * w)])

            # ---- geometry ----
            nc.vector.tensor_tensor(out=fl(dxyz, 3), in0=fl(posj, 3),
                                in1=fl(ctr, 3), op=TT.subtract)
            nc.vector.tensor_tensor(out=fl(sqt, 3), in0=fl(dxyz, 3),
                                    in1=fl(dxyz, 3), op=TT.mult)
            nc.vector.tensor_reduce(
                out=fl(r2, 1),
                in_=_ap(sqt, 0 * 3, [(3, CH), (1, 3)]),
                axis=mybir.AxisListType.X, op=TT.add)
            rr = tmp0  # scratch: r
            nc.scalar.sqrt(out=fl(rr, 1), in_=fl(r2, 1))
            nc.vector.reciprocal(out=fl(rinv, 1), in_=fl(rr, 1))
            nc.vector.tensor_tensor(
                out=fl(u, 3), in0=fl(dxyz, 3),
                in1=_ap(rinv, 0, [(1, CH), (0, 3)]), op=TT.mult)
            # cosx = cos(pi*r/R_C); xcp = xc+1 = 2(r/R_C-1)^2, via scalar
            nc.scalar.activation(out=fl(cosx, 1), in_=fl(rr, 1),
                                 func=AF.Sin, bias=math.pi / 2,
                                 scale=-math.pi / R_C)
            nc.scalar.activation(out=fl(s01, 1), in_=fl(rr, 1),
                                 func=AF.Square,
                                 scale=math.sqrt(2.0) / R_C,
                                 bias=-math.sqrt(2.0))
            nc.vector.tensor_scalar(out=fl(mask, 1), in0=fl(r2, 1),
                                scalar1=R_C * R_C, scalar2=None,
                                op0=TT.is_lt)
            nc.vector.tensor_scalar(out=fl(tmp0, 1), in0=fl(cosx, 1),
                                scalar1=0.25, scalar2=0.25,
                                op0=TT.mult, op1=TT.add)
            nc.vector.tensor_tensor(out=fl(fch, 1), in0=fl(tmp0, 1),
                                in1=fl(mask, 1), op=TT.mult)
            nc.vector.tensor_scalar(out=fl(x2, 1), in0=fl(s01, 1),
                                scalar1=2.0, scalar2=-2.0,
                                op0=TT.mult, op1=TT.add)


            # T_k stored k-major: Tall[k*CH:(k+1)*CH]; T0=1, T1=xc
            nc.vector.memset(_ap(Tall, 0, [(1, CH)]), 1.0)
            nc.vector.tensor_scalar(out=_ap(Tall, CH, [(1, CH)]),
                                    in0=fl(s01, 1), scalar1=-1.0,
                                    scalar2=None, op0=TT.add)
            nc.vector.scalar_tensor_tensor(
                out=_ap(Tall, 2 * CH, [(1, CH)]),
                in0=_ap(Tall, CH, [(1, CH)]), scalar=2.0,
                in1=_ap(Tall, CH, [(1, CH)]), op0=TT.mult, op1=TT.mult)
            nc.vector.tensor_scalar(out=_ap(Tall, 2 * CH, [(1, CH)]),
                                    in0=_ap(Tall, 2 * CH, [(1, CH)]),
                                    scalar1=-1.0, scalar2=None, op0=TT.add)
            for k in range(3, K_MAX):
                nc.vector.tensor_tensor(
                    out=_ap(Tall, k * CH, [(1, CH)]), in0=fl(x2, 1),
                    in1=_ap(Tall, (k - 1) * CH, [(1, CH)]), op=TT.mult)
                nc.vector.tensor_tensor(
                    out=_ap(Tall, k * CH, [(1, CH)]),
                    in0=_ap(Tall, k * CH, [(1, CH)]),
                    in1=_ap(Tall, (k - 2) * CH, [(1, CH)]), op=TT.subtract)
            # f[k, c] = (T_k + 1) * fch   (one flat op)
            nc.vector.scalar_tensor_tensor(
                out=_ap(f, 0, [(CH, K_MAX), (1, CH)]),
                in0=_ap(Tall, 0, [(CH, K_MAX), (1, CH)]), scalar=1.0,
                in1=_ap(fch, 0, [(0, K_MAX), (1, CH)]),
                op0=TT.add, op1=TT.mult)

            # ---- g path, k-major [k, d, c], all flat, in place -------------
            nc.vector.tensor_tensor(
                out=_ap(cpair, 0, [(8 * CH, K_MAX), (CH, 8), (1, CH)]),
                in0=_ap(cpair, 0, [(8 * CH, K_MAX), (CH, 8), (1, CH)]),
                in1=_ap(f, 0, [(CH, K_MAX), (0, 8), (1, CH)]),
                op=TT.mult)
            HKD = 4 * 8 * CH
            nc.vector.tensor_tensor(out=_ap(cpair, 0, [(1, HKD)]),
                                    in0=_ap(cpair, 0, [(1, HKD)]),
                                    in1=_ap(cpair, HKD, [(1, HKD)]),
                                    op=TT.add)
            nc.vector.tensor_tensor(out=_ap(cpair, 0, [(1, HKD // 2)]),
                                    in0=_ap(cpair, 0, [(1, HKD // 2)]),
                                    in1=_ap(cpair, HKD // 2, [(1, HKD // 2)]),
                                    op=TT.add)
            nc.vector.tensor_tensor(out=_ap(g, 0, [(1, 8 * CH)]),
                                    in0=_ap(cpair, 0, [(1, 8 * CH)]),
                                    in1=_ap(cpair, 8 * CH, [(1, 8 * CH)]),
                                    op=TT.add)
            # g layout [d, c]
            nc.vector.tensor_tensor(out=_ap(gsq, 0, [(1, 8 * CH)]),
                                    in0=_ap(g, 0, [(1, 8 * CH)]),
                                    in1=_ap(g, 0, [(1, 8 * CH)]),
                                    op=TT.mult)
            for (cw0, cwn) in ((0, 106), (106, 104)):
                # gT[c, d] <- g[d, c] (small 1x transpose copy)
                nc.vector.tensor_copy(
                    out=_ap(gT, cw0 * 8, [(8, cwn), (1, 8)]),
                    in_=_ap(g, cw0, [(1, cwn), (CH, 8)]))
                # W[(j,a), c, (a',d)] = gT * (a' == a), flat 2x
                nc.vector.tensor_tensor(
                    out=_ap(W, cw0 * 48, [(48, cwn), (8, 6), (1, 8)]),
                    in0=_ap(gT, cw0 * 8, [(8, cwn), (0, 6), (1, 8)]),
                    in1=_ap(mk, 0, [(0, cwn), (8, 6), (1, 8)]),
                    op=TT.mult)

            def emit_half(h):
                c0 = h * HB

                def sl(t, w):  # flat [c0*w, HB*w) slice AP
                    return _ap(t, c0 * w, [(1, HB * w)])

                # ---- spherical harmonics Y[16] (gpsimd) ----
                def y_slice(m, cnt=1):
                    return _ap(Y, c0 * 16 + m, [(16, HB), (1, cnt)])

                def u_c(c):
                    return _ap(u, c0 * 3 + c, [(3, HB)])

                gp = nc.gpsimd
                nc.scalar.activation(out=y_slice(1, 3),
                                     in_=_ap(u, c0 * 3, [(3, HB), (1, 3)]),
                                     func=AF.Copy)
                gp.tensor_tensor(out=sl(x2c, 1), in0=u_c(0), in1=u_c(0),
                                 op=TT.mult)
                gp.tensor_tensor(out=sl(y2c, 1), in0=u_c(1), in1=u_c(1),
                                 op=TT.mult)
                gp.tensor_tensor(out=sl(z2c, 1), in0=u_c(2), in1=u_c(2),
                                 op=TT.mult)
                # scalar-engine pre-scales so gpsimd only needs tensor_tensor
                nc.scalar.activation(out=sl(uzs, 1), in_=u_c(2), func=AF.Copy,
                                     scale=SQ3)
                nc.scalar.activation(out=sl(uzC, 1), in_=u_c(2), func=AF.Copy,
                                     scale=C32)
                nc.scalar.activation(out=sl(y3c, 1), in_=sl(y2c, 1),
                                     func=AF.Copy, scale=3.0)
                nc.scalar.activation(out=sl(x3c, 1), in_=sl(x2c, 1),
                                     func=AF.Copy, scale=3.0)
                nc.scalar.activation(out=sl(uxC, 1), in_=u_c(0), func=AF.Copy,
                                     scale=-C33)
                nc.scalar.activation(out=sl(uyC, 1), in_=u_c(1), func=AF.Copy,
                                     scale=C33)
                gp.tensor_tensor(out=sl(xyc, 1), in0=u_c(0), in1=u_c(1),
                                 op=TT.mult)
                nc.scalar.activation(out=y_slice(4), in_=sl(xyc, 1),
                                     func=AF.Copy, scale=SQ3)
                gp.tensor_tensor(out=y_slice(5), in0=u_c(1), in1=sl(uzs, 1),
                                 op=TT.mult)
                gp.tensor_tensor(out=y_slice(6), in0=u_c(0), in1=sl(uzs, 1),
                                 op=TT.mult)
                nc.scalar.activation(out=y_slice(7), in_=sl(z2c, 1),
                                     func=AF.Copy, scale=1.5, bias=-0.5)
                gp.tensor_tensor(out=sl(dxyc, 1), in0=sl(x2c, 1),
                                 in1=sl(y2c, 1), op=TT.subtract)
                nc.scalar.activation(out=y_slice(8), in_=sl(dxyc, 1),
                                     func=AF.Copy, scale=SQ3 / 2)
                nc.scalar.activation(out=sl(dxyh, 1), in_=sl(dxyc, 1),
                                     func=AF.Copy, scale=0.5)
                nc.scalar.activation(out=sl(tl3, 1), in_=sl(z2c, 1),
                                     func=AF.Copy, scale=2.5, bias=-1.5)
                gp.tensor_tensor(out=y_slice(9), in0=sl(tl3, 1), in1=u_c(2),
                                 op=TT.mult)
                nc.scalar.activation(out=sl(tl4, 1), in_=sl(z2c, 1),
                                     func=AF.Copy, scale=5.0 * C31,
                                     bias=-C31)
                gp.tensor_tensor(out=y_slice(10), in0=sl(tl4, 1), in1=u_c(0),
                                 op=TT.mult)
                gp.tensor_tensor(out=y_slice(11), in0=sl(tl4, 1), in1=u_c(1),
                                 op=TT.mult)
                gp.tensor_tensor(out=y_slice(12), in0=sl(dxyh, 1),
                                 in1=sl(uzC, 1), op=TT.mult)
                gp.tensor_tensor(out=y_slice(13), in0=sl(xyc, 1),
                                 in1=sl(uzC, 1), op=TT.mult)
                gp.tensor_tensor(out=sl(tl5, 1), in0=sl(y3c, 1),
                                 in1=sl(x2c, 1), op=TT.subtract)
                gp.tensor_tensor(out=y_slice(14), in0=sl(tl5, 1),
                                 in1=sl(uxC, 1), op=TT.mult)
                gp.tensor_tensor(out=sl(tl6, 1), in0=sl(x3c, 1),
                                 in1=sl(y2c, 1), op=TT.subtract)
                gp.tensor_tensor(out=y_slice(15), in0=sl(tl6, 1),
                                 in1=sl(uyC, 1), op=TT.mult)
                # Y16 = Y * sqrt(0.5), bf16 (folds the 0.5 into A^2)
                nc.scalar.activation(
                    out=_ap(Y16, c0 * 16 + 1, [(16, HB), (1, 15)]),
                    in_=_ap(Y, c0 * 16 + 1, [(16, HB), (1, 15)]),
                    func=AF.Copy, scale=SQH)

                # ---- tensor engine: A and B contractions over j ----
                for grp in range(HB // CGRP):
                    cg = c0 + grp * CGRP
                    pt = psA.tile([128, CGRP * 16], F32, tag="psA",
                                  name="psA")
                    for ci in range(CGRP):
                        c = c0 + grp * CGRP + ci
                        nc.tensor.matmul(
                            pt[0:48, ci * 16:(ci + 1) * 16],
                            _ap(W, c * 48, [(1, 48)]),
                            _ap(Y16, c * 16, [(1, 16)]),
                            start=True, stop=True)
                    gi = h * (HB // CGRP) + grp
                    nc.scalar.copy(out=Araw[0:48, gi * 336:(gi + 1) * 336],
                                   in_=pt[0:48, 0:336])
                    nc.scalar.square(
                        out=Araw[0:48, gi * 336:(gi + 1) * 336],
                        in_=Araw[0:48, gi * 336:(gi + 1) * 336])
                    if gi % 2 == 1:
                        c2 = (gi - 1) * CGRP
                        for l in range(L_MAX):
                            cnt = SHELL_OFF[l + 1] - SHELL_OFF[l]
                            nc.vector.tensor_reduce(
                                out=_ap(qa, c2 * L_MAX + l,
                                        [(L_MAX, 2 * CGRP)], parts=48),
                                in_=_ap(Araw, c2 * 16 + SHELL_OFF[l],
                                        [(16, 2 * CGRP), (1, cnt)],
                                        parts=48),
                                axis=mybir.AxisListType.X, op=TT.add)
            def emit_tail(h):
                c0 = h * HB
                nc.sync.dma_start(
                    out=qa_d[:, c0 * L_MAX:(c0 + HB) * L_MAX],
                    in_=qa[0:48, c0 * L_MAX:(c0 + HB) * L_MAX])
                nc.sync.dma_start(out=braw_d[:, h * 840:(h + 1) * 840],
                                  in_=braw[0:A6, h * 840:(h + 1) * 840])

            def emit_bh(h):
                c0 = h * HB
                for dg in range(2):
                    ptb = psB.tile([128, 4 * HB], F32, tag="psB", name="psB")
                    for dd in range(4):
                        d = dg * 4 + dd
                        nc.tensor.matmul(
                            ptb[0:A6, dd * HB:(dd + 1) * HB],
                            wones[0:PP, 0:A6],
                            _ap(gsq, d * CH + c0, [(1, HB)]),
                            start=True, stop=True)
                    nc.scalar.copy(
                        out=braw[0:A6,
                                 h * 840 + dg * 420:h * 840 + (dg + 1) * 420],
                        in_=ptb[0:A6, 0:420])


            emit_half(0)
            emit_half(1)
            emit_bh(0)
            emit_bh(1)
            emit_tail(0)
            emit_tail(1)

            if debug:
                for nm, t in [("d_g", g), ("d_Y", Y), ("d_W", W),
                              ("d_A", Araw), ("d_f", f), ("d_u", u)]:
                    dd = nc.declare_dram_parameter(
                        nm, [128, t.shape[1]], F32, isOutput=True)
                    nc.sync.dma_start(out=dd[:], in_=t[:])
    nc.finalize()
    return nc


def make_inputs(types, positions, angular_neighbors, c_table):
    """Host-side marshaling: dense per-core slabs, pairs-on-partitions."""
    import ml_dtypes
    types = np.asarray(types).astype(np.int64)
    positions = np.ascontiguousarray(np.asarray(positions, dtype=np.float32))
    nbr = np.asarray(angular_neighbors).astype(np.int64)
    c_table = np.asarray(c_table, dtype=np.float32)

    pad = NTOT - N_ATOMS
    types_pad = np.concatenate([types, np.repeat(types[-1:], pad, 0)], 0)
    pos_pad = np.concatenate([positions, np.repeat(positions[-1:], pad, 0)], 0)
    nbr_pad = np.concatenate([nbr, np.repeat(nbr[-1:], pad, 0)], 0)

    aI = np.arange(A6)
    cI = np.arange(CH)
    mk = np.zeros((M_NBR, A6, A6, N_DESC), dtype=np.float32)
    mk[:, aI, aI, :] = 1.0
    mk = mk.reshape(PP, 48).astype(ml_dtypes.bfloat16)
    wones = np.zeros((M_NBR, A6, A6), dtype=np.float32)
    wones[:, aI, aI] = 1.0
    wones = wones.reshape(PP, A6).astype(ml_dtypes.bfloat16)

    in_maps = []
    for core in range(NCORES):
        at = core * CA + cI[None, :] * A6 + aI[:, None]        # [A6, CH]
        nb = nbr_pad[at].transpose(2, 0, 1).reshape(PP, CH)    # [(j,a), c]
        posj = pos_pad[nb].reshape(PP, CH * 3)
        ctr = np.broadcast_to(pos_pad[at], (M_NBR, A6, CH, 3)
                              ).reshape(PP, CH * 3)
        ti = np.broadcast_to(types_pad[at], (M_NBR, A6, CH)).reshape(PP, CH)
        tj = types_pad[nb]
        cpair = np.ascontiguousarray(
            c_table[ti, tj].transpose(0, 3, 2, 1)).reshape(PP, CH * 64)
        def p128(a):
            out = np.zeros((128, a.shape[1]), dtype=a.dtype)
            out[:PP] = a
            return out

        in_maps.append({
            "posj": p128(np.ascontiguousarray(posj)),
            "ctr": p128(np.ascontiguousarray(ctr)),
            "cpair": p128(np.ascontiguousarray(cpair).astype(
                ml_dtypes.bfloat16)),
            "mk": p128(mk),
            "wones": p128(wones),
        })
    return in_maps


_NC_CACHE = None


def kernel(types, positions, angular_neighbors, c_table):
    global _NC_CACHE
    in_maps = make_inputs(types, positions, angular_neighbors, c_table)
    if _NC_CACHE is None:
        _NC_CACHE = build_nc()
    res = run_bass_kernel_spmd(_NC_CACHE, in_maps,
                               core_ids=list(range(NCORES)))
    outs = []
    for core in range(NCORES):
        qa = res.results[core]["qa"].reshape(A6, N_DESC, CH, L_MAX)
        braw = res.results[core]["braw"].reshape(A6, 2, N_DESC, HB)
        B = np.concatenate([braw[:, 0], braw[:, 1]], axis=-1)  # [A6, 8, CH]
        q = qa - 0.5 * B[..., None]                  # [a, d, c, l]
        outs.append(q.transpose(2, 0, 1, 3).reshape(CA, N_DESC, L_MAX))
    q = np.concatenate(outs, 0)[:N_ATOMS]
    return np.ascontiguousarray(q.astype(np.float32))


if __name__ == "__main__":
    if os.path.exists("/tmp/ref_cache.npz"):
        z = np.load("/tmp/ref_cache.npz")
        inputs = {k: z[k] for k in
                  ("types", "positions", "angular_neighbors", "c_table")}
        exp = z["exp"]
    else:
        import reference
        inputs = {k: np.asarray(v) for k, v in reference.setup_inputs().items()}
        exp = np.asarray(reference.reference(**inputs))
    act = kernel(**inputs)
    rel = np.linalg.norm(act - exp) / np.linalg.norm(exp)
    print("Relative error:", rel)


# revision 29
# speedup vs baseline: 1.0324x; 1.0324x over previous
"""Trainium2 Bass kernel for nn_AngularDescriptor (gnn_message_passing).

Legendre addition theorem factorization:
  q[i,d,l] = 0.5 * ( sum_{m in shell l} A[i,d,m]^2  -  B[i,d] )
  A[i,d,m] = sum_j g_ij[d] * Yhat_m(u_ij),   B[i,d] = sum_j g_ij[d]^2

v4: pairs-on-partitions layout.  Partition p = j*6 + a (j = neighbor
0..19, a = atom lane 0..5), free dim = 210 atom-chunks per core.  All
index gathers are host-marshaled dense slabs.  The A and B contractions
over j run on the tensor engine: per chunk a block-diagonal weight
matrix W[(j,a),(a',d)] = g * (a==a') multiplies Y -> PSUM A[(a,d), m];
B uses a fixed block-ones lhsT.  The 0.5*q - 0.5*B combine and the
output unscramble happen on host (pure reshapes + one subtract).
"""
import os
import sys

sys.path.insert(0, "/opt/trn_rl_repo")
os.environ.setdefault("NEURON_RT_RESET_CORES", "1")

import math
import numpy as np

from concourse import bacc, bass, mybir, tile
from concourse.bass_utils import run_bass_kernel_spmd

N_ATOMS = 10000
M_NBR = 20
N_TYPES = 4
N_DESC = 8
K_MAX = 8
L_MAX = 4
R_C = 5.0

NCORES = 8
A6 = 6                      # atom lanes per partition group
PP = M_NBR * A6             # 120 active partitions
CH = 210                    # atom chunks per core
HB = 105                    # chunks per half (software pipeline)
CGRP = 21                   # chunks per PSUM group (21*16 = 336 cols)
CA = A6 * CH                # atoms per core = 1260
NTOT = NCORES * CA          # padded atom count = 10080
F32 = mybir.dt.float32
BF16 = mybir.dt.bfloat16

SQ3 = math.sqrt(3.0)
C31 = math.sqrt(3.0 / 8.0)
C32 = math.sqrt(15.0)
C33 = math.sqrt(5.0 / 8.0)
SHELL_OFF = [0, 1, 4, 9, 16]
SQH = math.sqrt(0.5)


def _ap(t, off, dims, parts=PP):
    """Custom free-dim AP on a tile: dims = [(step, count), ...]."""
    base = t[:]
    ap = [[base.ap[0][0], parts]] + [[s, c] for (s, c) in dims]
    return bass.AP(base.tensor, base.offset + off, ap)


def build_nc(debug=False):
    nc = bacc.Bacc()
    posj_d = nc.declare_dram_parameter("posj", [128, CH * 3], F32,
                                       isOutput=False)
    ctr_d = nc.declare_dram_parameter("ctr", [128, CH * 3], F32,
                                      isOutput=False)
    cpair_d = nc.declare_dram_parameter("cpair", [128, CH * 64], BF16,
                                        isOutput=False)
    mk_d = nc.declare_dram_parameter("mk", [128, 48], BF16, isOutput=False)
    wones_d = nc.declare_dram_parameter("wones", [128, A6], BF16,
                                        isOutput=False)
    qa_d = nc.declare_dram_parameter("qa", [48, CH * L_MAX], F32,
                                     isOutput=True)
    braw_d = nc.declare_dram_parameter("braw", [A6, 2 * N_DESC * HB], F32,
                                       isOutput=True)

    with tile.TileContext(nc) as tc:
        with tc.tile_pool(name="main", bufs=1) as pool, \
             tc.tile_pool(name="psA", bufs=3, space="PSUM") as psA, \
             tc.tile_pool(name="psB", bufs=2, space="PSUM") as psB:
            consts = pool.tile([128, 2], F32)
            nc.vector.memset(consts[:, 0:1], math.pi / 2)
            nc.const_aps.aps[(F32, math.pi / 2)] = consts[:, 0:1]
            nc.vector.memset(consts[:, 1:2], -math.sqrt(2.0))
            nc.const_aps.aps[(F32, -math.sqrt(2.0))] = consts[:, 1:2]

            posj = pool.tile([128, CH * 3], F32)
            ctr = pool.tile([128, CH * 3], F32)
            cpair = pool.tile([128, CH * 64], BF16)   # becomes gtmp/t1
            mk = pool.tile([128, 48], BF16)
            wones = pool.tile([128, A6], BF16)
            gT = pool.tile([128, CH * N_DESC], BF16)
            nc.sync.dma_start(out=posj[:], in_=posj_d[:])
            nc.sync.dma_start(out=ctr[:], in_=ctr_d[:])
            nc.sync.dma_start(out=wones[:], in_=wones_d[:])
            nc.sync.dma_start(out=mk[:], in_=mk_d[:])
            for qq in range(4):
                cs = slice(qq * (CH * 16), (qq + 1) * (CH * 16))
                nc.sync.dma_start(out=cpair[:, cs], in_=cpair_d[:, cs])

            dxyz = pool.tile([128, CH * 3], F32)
            sqt = pool.tile([128, CH * 3], F32)
            r2 = pool.tile([128, CH], F32)
            rinv = pool.tile([128, CH], F32)
            u = pool.tile([128, CH * 3], F32)
            s01 = pool.tile([128, CH], F32)
            cosx = pool.tile([128, CH], F32)
            mask = pool.tile([128, CH], F32)
            fch = pool.tile([128, CH], F32)
            tmp0 = pool.tile([128, CH], F32)
            tm1 = pool.tile([128, CH], F32)
            xc = pool.tile([128, CH], F32)
            x2 = pool.tile([128, CH], F32)
            Tall = pool.tile([128, CH * K_MAX], F32)
            f = pool.tile([128, CH * K_MAX], BF16)
            g = pool.tile([128, CH * N_DESC], BF16)
            gsq = pool.tile([128, CH * N_DESC], BF16)
            W = pool.tile([128, CH * 48], BF16)
            Y = pool.tile([128, CH * 16], F32)
            Y16 = pool.tile([128, CH * 16], BF16)
            x2c = pool.tile([128, CH], F32)
            y2c = pool.tile([128, CH], F32)
            z2c = pool.tile([128, CH], F32)
            xyc = pool.tile([128, CH], F32)
            dxyc = pool.tile([128, CH], F32)
            tl3 = pool.tile([128, CH], F32)
            tl4 = pool.tile([128, CH], F32)
            tl5 = pool.tile([128, CH], F32)
            tl6 = pool.tile([128, CH], F32)
            uzs = pool.tile([128, CH], F32)
            uzC = pool.tile([128, CH], F32)
            dxyh = pool.tile([128, CH], F32)
            y3c = pool.tile([128, CH], F32)
            x3c = pool.tile([128, CH], F32)
            uxC = pool.tile([128, CH], F32)
            uyC = pool.tile([128, CH], F32)
            Araw = pool.tile([128, CH * 16], F32)
            qa = pool.tile([128, CH * L_MAX], F32)
            braw = pool.tile([128, 2 * N_DESC * HB], F32)

            TT = mybir.AluOpType
            AF = mybir.ActivationFunctionType

            # Y16[m=0] = sqrt(0.5); slots m>=1 overwritten by the convert
            nc.gpsimd.memset(Y16[0:PP, :], SQH)

            def fl(t, w):
                return _ap(t, 0, [(1, CH * w)])

            # ---- geometry ----
            nc.vector.tensor_tensor(out=fl(dxyz, 3), in0=fl(posj, 3),
                                in1=fl(ctr, 3), op=TT.subtract)
            nc.vector.tensor_tensor(out=fl(sqt, 3), in0=fl(dxyz, 3),
                                    in1=fl(dxyz, 3), op=TT.mult)
            nc.vector.tensor_reduce(
                out=fl(r2, 1),
                in_=_ap(sqt, 0 * 3, [(3, CH), (1, 3)]),
                axis=mybir.AxisListType.X, op=TT.add)
            rr = tmp0  # scratch: r
            nc.scalar.sqrt(out=fl(rr, 1), in_=fl(r2, 1))
            nc.vector.reciprocal(out=fl(rinv, 1), in_=fl(rr, 1))
            nc.vector.tensor_tensor(
                out=fl(u, 3), in0=fl(dxyz, 3),
                in1=_ap(rinv, 0, [(1, CH), (0, 3)]), op=TT.mult)
            # cosx = cos(pi*r/R_C); xcp = xc+1 = 2(r/R_C-1)^2, via scalar
            nc.scalar.activation(out=fl(cosx, 1), in_=fl(rr, 1),
                                 func=AF.Sin, bias=math.pi / 2,
                                 scale=-math.pi / R_C)
            nc.scalar.activation(out=fl(s01, 1), in_=fl(rr, 1),
                                 func=AF.Square,
                                 scale=math.sqrt(2.0) / R_C,
                                 bias=-math.sqrt(2.0))
            nc.vector.tensor_scalar(out=fl(mask, 1), in0=fl(r2, 1),
                                scalar1=R_C * R_C, scalar2=None,
                                op0=TT.is_lt)
            nc.vector.tensor_scalar(out=fl(tmp0, 1), in0=fl(cosx, 1),
                                scalar1=0.25, scalar2=0.25,
                                op0=TT.mult, op1=TT.add)
            nc.vector.tensor_tensor(out=fl(fch, 1), in0=fl(tmp0, 1),
                                in1=fl(mask, 1), op=TT.mult)
            nc.vector.tensor_scalar(out=fl(x2, 1), in0=fl(s01, 1),
                                scalar1=2.0, scalar2=-2.0,
                                op0=TT.mult, op1=TT.add)


            # T_k stored k-major: Tall[k*CH:(k+1)*CH]; T0=1, T1=xc
            nc.vector.memset(_ap(Tall, 0, [(1, CH)]), 1.0)
            nc.vector.tensor_scalar(out=_ap(Tall, CH, [(1, CH)]),
                                    in0=fl(s01, 1), scalar1=-1.0,
                                    scalar2=None, op0=TT.add)
            nc.vector.scalar_tensor_tensor(
                out=_ap(Tall, 2 * CH, [(1, CH)]),
                in0=_ap(Tall, CH, [(1, CH)]), scalar=2.0,
                in1=_ap(Tall, CH, [(1, CH)]), op0=TT.mult, op1=TT.mult)
            nc.vector.tensor_scalar(out=_ap(Tall, 2 * CH, [(1, CH)]),
                                    in0=_ap(Tall, 2 * CH, [(1, CH)]),
                                    scalar1=-1.0, scalar2=None, op0=TT.add)
            for k in range(3, K_MAX):
                nc.vector.tensor_tensor(
                    out=_ap(Tall, k * CH, [(1, CH)]), in0=fl(x2, 1),
                    in1=_ap(Tall, (k - 1) * CH, [(1, CH)]), op=TT.mult)
                nc.vector.tensor_tensor(
                    out=_ap(Tall, k * CH, [(1, CH)]),
                    in0=_ap(Tall, k * CH, [(1, CH)]),
                    in1=_ap(Tall, (k - 2) * CH, [(1, CH)]), op=TT.subtract)
            # f[k, c] = (T_k + 1) * fch   (one flat op)
            nc.vector.scalar_tensor_tensor(
                out=_ap(f, 0, [(CH, K_MAX), (1, CH)]),
                in0=_ap(Tall, 0, [(CH, K_MAX), (1, CH)]), scalar=1.0,
                in1=_ap(fch, 0, [(0, K_MAX), (1, CH)]),
                op0=TT.add, op1=TT.mult)

            # ---- g path, k-major [k, d, c], all flat, in place -------------
            nc.vector.tensor_tensor(
                out=_ap(cpair, 0, [(8 * CH, K_MAX), (CH, 8), (1, CH)]),
                in0=_ap(cpair, 0, [(8 * CH, K_MAX), (CH, 8), (1, CH)]),
                in1=_ap(f, 0, [(CH, K_MAX), (0, 8), (1, CH)]),
                op=TT.mult)
            HKD = 4 * 8 * CH
            nc.vector.tensor_tensor(out=_ap(cpair, 0, [(1, HKD)]),
                                    in0=_ap(cpair, 0, [(1, HKD)]),
                                    in1=_ap(cpair, HKD, [(1, HKD)]),
                                    op=TT.add)
            nc.vector.tensor_tensor(out=_ap(cpair, 0, [(1, HKD // 2)]),
                                    in0=_ap(cpair, 0, [(1, HKD // 2)]),
                                    in1=_ap(cpair, HKD // 2, [(1, HKD // 2)]),
                                    op=TT.add)
            nc.vector.tensor_tensor(out=_ap(g, 0, [(1, 8 * CH)]),
                                    in0=_ap(cpair, 0, [(1, 8 * CH)]),
                                    in1=_ap(cpair, 8 * CH, [(1, 8 * CH)]),
                                    op=TT.add)
            # g layout [d, c]
            nc.vector.tensor_tensor(out=_ap(gsq, 0, [(1, 8 * CH)]),
                                    in0=_ap(g, 0, [(1, 8 * CH)]),
                                    in1=_ap(g, 0, [(1, 8 * CH)]),
                                    op=TT.mult)
            for (cw0, cwn) in ((0, 106), (106, 104)):
                # gT[c, d] <- g[d, c] (small 1x transpose copy)
                nc.vector.tensor_copy(
                    out=_ap(gT, cw0 * 8, [(8, cwn), (1, 8)]),
                    in_=_ap(g, cw0, [(1, cwn), (CH, 8)]))
                # W[(j,a), c, (a',d)] = gT * (a' == a), flat 2x
                nc.vector.tensor_tensor(
                    out=_ap(W, cw0 * 48, [(48, cwn), (8, 6), (1, 8)]),
                    in0=_ap(gT, cw0 * 8, [(8, cwn), (0, 6), (1, 8)]),
                    in1=_ap(mk, 0, [(0, cwn), (8, 6), (1, 8)]),
                    op=TT.mult)

            def emit_half(h):
                c0 = h * HB

                def sl(t, w):  # flat [c0*w, HB*w) slice AP
                    return _ap(t, c0 * w, [(1, HB * w)])

                # ---- spherical harmonics Y[16] (gpsimd) ----
                def y_slice(m, cnt=1):
                    return _ap(Y, c0 * 16 + m, [(16, HB), (1, cnt)])

                def u_c(c):
                    return _ap(u, c0 * 3 + c, [(3, HB)])

                gp = nc.gpsimd
                nc.scalar.activation(out=y_slice(1, 3),
                                     in_=_ap(u, c0 * 3, [(3, HB), (1, 3)]),
                                     func=AF.Copy)
                gp.tensor_tensor(out=sl(x2c, 1), in0=u_c(0), in1=u_c(0),
                                 op=TT.mult)
                gp.tensor_tensor(out=sl(y2c, 1), in0=u_c(1), in1=u_c(1),
                                 op=TT.mult)
                gp.tensor_tensor(out=sl(z2c, 1), in0=u_c(2), in1=u_c(2),
                                 op=TT.mult)
                # scalar-engine pre-scales so gpsimd only needs tensor_tensor
                nc.scalar.activation(out=sl(uzs, 1), in_=u_c(2), func=AF.Copy,
                                     scale=SQ3)
                nc.scalar.activation(out=sl(uzC, 1), in_=u_c(2), func=AF.Copy,
                                     scale=C32)
                nc.scalar.activation(out=sl(y3c, 1), in_=sl(y2c, 1),
                                     func=AF.Copy, scale=3.0)
                nc.scalar.activation(out=sl(x3c, 1), in_=sl(x2c, 1),
                                     func=AF.Copy, scale=3.0)
                nc.scalar.activation(out=sl(uxC, 1), in_=u_c(0), func=AF.Copy,
                                     scale=-C33)
                nc.scalar.activation(out=sl(uyC, 1), in_=u_c(1), func=AF.Copy,
                                     scale=C33)
                gp.tensor_tensor(out=sl(xyc, 1), in0=u_c(0), in1=u_c(1),
                                 op=TT.mult)
                nc.scalar.activation(out=y_slice(4), in_=sl(xyc, 1),
                                     func=AF.Copy, scale=SQ3)
                gp.tensor_tensor(out=y_slice(5), in0=u_c(1), in1=sl(uzs, 1),
                                 op=TT.mult)
                gp.tensor_tensor(out=y_slice(6), in0=u_c(0), in1=sl(uzs, 1),
                                 op=TT.mult)
                nc.scalar.activation(out=y_slice(7), in_=sl(z2c, 1),
                                     func=AF.Copy, scale=1.5, bias=-0.5)
                gp.tensor_tensor(out=sl(dxyc, 1), in0=sl(x2c, 1),
                                 in1=sl(y2c, 1), op=TT.subtract)
                nc.scalar.activation(out=y_slice(8), in_=sl(dxyc, 1),
                                     func=AF.Copy, scale=SQ3 / 2)
                nc.scalar.activation(out=sl(dxyh, 1), in_=sl(dxyc, 1),
                                     func=AF.Copy, scale=0.5)
                nc.scalar.activation(out=sl(tl3, 1), in_=sl(z2c, 1),
                                     func=AF.Copy, scale=2.5, bias=-1.5)
                gp.tensor_tensor(out=y_slice(9), in0=sl(tl3, 1), in1=u_c(2),
                                 op=TT.mult)
                nc.scalar.activation(out=sl(tl4, 1), in_=sl(z2c, 1),
                                     func=AF.Copy, scale=5.0 * C31,
                                     bias=-C31)
                gp.tensor_tensor(out=y_slice(10), in0=sl(tl4, 1), in1=u_c(0),
                                 op=TT.mult)
                gp.tensor_tensor(out=y_slice(11), in0=sl(tl4, 1), in1=u_c(1),
                                 op=TT.mult)
                gp.tensor_tensor(out=y_slice(12), in0=sl(dxyh, 1),
                                 in1=sl(uzC, 1), op=TT.mult)
                gp.tensor_tensor(out=y_slice(13), in0=sl(xyc, 1),
                                 in1=sl(uzC, 1), op=TT.mult)
                gp.tensor_tensor(out=sl(tl5, 1), in0=sl(y3c, 1),
                                 in1=sl(x2c, 1), op=TT.subtract)
                gp.tensor_tensor(out=y_slice(14), in0=sl(tl5, 1),
                                 in1=sl(uxC, 1), op=TT.mult)
                gp.tensor_tensor(out=sl(tl6, 1), in0=sl(x3c, 1),
                                 in1=sl(y2c, 1), op=TT.subtract)
                gp.tensor_tensor(out=y_slice(15), in0=sl(tl6, 1),
                                 in1=sl(uyC, 1), op=TT.mult)
                # Y16 = Y * sqrt(0.5), bf16 (folds the 0.5 into A^2)
                nc.scalar.activation(
                    out=_ap(Y16, c0 * 16 + 1, [(16, HB), (1, 15)]),
                    in_=_ap(Y, c0 * 16 + 1, [(16, HB), (1, 15)]),
                    func=AF.Copy, scale=SQH)

                # ---- tensor engine: A and B contractions over j ----
                for grp in range(HB // CGRP):
                    cg = c0 + grp * CGRP
                    pt = psA.tile([128, CGRP * 16], F32, tag="psA",
                                  name="psA")
                    for ci in range(CGRP):
                        c = c0 + grp * CGRP + ci
                        nc.tensor.matmul(
                            pt[0:48, ci * 16:(ci + 1) * 16],
                            _ap(W, c * 48, [(1, 48)]),
                            _ap(Y16, c * 16, [(1, 16)]),
                            start=True, stop=True)
                    gi = h * (HB // CGRP) + grp
                    nc.scalar.copy(out=Araw[0:48, gi * 336:(gi + 1) * 336],
                                   in_=pt[0:48, 0:336])
                    nc.scalar.square(
                        out=Araw[0:48, gi * 336:(gi + 1) * 336],
                        in_=Araw[0:48, gi * 336:(gi + 1) * 336])
                    if gi % 2 == 1:
                        c2 = (gi - 1) * CGRP
                        for l in range(L_MAX):
                            cnt = SHELL_OFF[l + 1] - SHELL_OFF[l]
                            nc.vector.tensor_reduce(
                                out=_ap(qa, c2 * L_MAX + l,
                                        [(L_MAX, 2 * CGRP)], parts=48),
                                in_=_ap(Araw, c2 * 16 + SHELL_OFF[l],
                                        [(16, 2 * CGRP), (1, cnt)],
                                        parts=48),
                                axis=mybir.AxisListType.X, op=TT.add)
            def emit_tail(h):
                c0 = h * HB
                nc.sync.dma_start(
                    out=qa_d[:, c0 * L_MAX:(c0 + HB) * L_MAX],
                    in_=qa[0:48, c0 * L_MAX:(c0 + HB) * L_MAX])
                nc.sync.dma_start(out=braw_d[:, h * 840:(h + 1) * 840],
                                  in_=braw[0:A6, h * 840:(h + 1) * 840])

            def emit_bh(h):
                c0 = h * HB
                for dg in range(2):
                    ptb = psB.tile([128, 4 * HB], F32, tag="psB", name="psB")
                    for dd in range(4):
                        d = dg * 4 + dd
                        nc.tensor.matmul(
                            ptb[0:A6, dd * HB:(dd + 1) * HB],
                            wones[0:PP, 0:A6],
                            _ap(gsq, d * CH + c0, [(1, HB)]),
                            start=True, stop=True)
                    nc.scalar.copy(
                        out=braw[0:A6,
                                 h * 840 + dg * 420:h * 840 + (dg + 1) * 420],
                        in_=ptb[0:A6, 0:420])


            emit_half(0)
            emit_half(1)
            emit_bh(0)
            emit_bh(1)
            emit_tail(0)
            emit_tail(1)

            if debug:
                for nm, t in [("d_g", g), ("d_Y", Y), ("d_W", W),
                              ("d_A", Araw), ("d_f", f), ("d_u", u)]:
                    dd = nc.declare_dram_parameter(
                        nm, [128, t.shape[1]], F32, isOutput=True)
                    nc.sync.dma_start(out=dd[:], in_=t[:])
    nc.finalize()
    return nc


def make_inputs(types, positions, angular_neighbors, c_table):
    """Host-side marshaling: dense per-core slabs, pairs-on-partitions."""
    import ml_dtypes
    types = np.asarray(types).astype(np.int64)
    positions = np.ascontiguousarray(np.asarray(positions, dtype=np.float32))
    nbr = np.asarray(angular_neighbors).astype(np.int64)
    c_table = np.asarray(c_table, dtype=np.float32)

    pad = NTOT - N_ATOMS
    types_pad = np.concatenate([types, np.repeat(types[-1:], pad, 0)], 0)
    pos_pad = np.concatenate([positions, np.repeat(positions[-1:], pad, 0)], 0)
    nbr_pad = np.concatenate([nbr, np.repeat(nbr[-1:], pad, 0)], 0)

    aI = np.arange(A6)
    cI = np.arange(CH)
    mk = np.zeros((M_NBR, A6, A6, N_DESC), dtype=np.float32)
    mk[:, aI, aI, :] = 1.0
    mk = mk.reshape(PP, 48).astype(ml_dtypes.bfloat16)
    wones = np.zeros((M_NBR, A6, A6), dtype=np.float32)
    wones[:, aI, aI] = 1.0
    wones = wones.reshape(PP, A6).astype(ml_dtypes.bfloat16)

    in_maps = []
    for core in range(NCORES):
        at = core * CA + cI[None, :] * A6 + aI[:, None]        # [A6, CH]
        nb = nbr_pad[at].transpose(2, 0, 1).reshape(PP, CH)    # [(j,a), c]
        posj = pos_pad[nb].reshape(PP, CH * 3)
        ctr = np.broadcast_to(pos_pad[at], (M_NBR, A6, CH, 3)
                              ).reshape(PP, CH * 3)
        ti = np.broadcast_to(types_pad[at], (M_NBR, A6, CH)).reshape(PP, CH)
        tj = types_pad[nb]
        cpair = np.ascontiguousarray(
            c_table[ti, tj].transpose(0, 3, 2, 1)).reshape(PP, CH * 64)
        def p128(a):
            out = np.zeros((128, a.shape[1]), dtype=a.dtype)
            out[:PP] = a
            return out

        in_maps.append({
            "posj": p128(np.ascontiguousarray(posj)),
            "ctr": p128(np.ascontiguousarray(ctr)),
            "cpair": p128(np.ascontiguousarray(cpair).astype(
                ml_dtypes.bfloat16)),
            "mk": p128(mk),
            "wones": p128(wones),
        })
    return in_maps


_NC_CACHE = None


def kernel(types, positions, angular_neighbors, c_table):
    global _NC_CACHE
    in_maps = make_inputs(types, positions, angular_neighbors, c_table)
    if _NC_CACHE is None:
        _NC_CACHE = build_nc()
    res = run_bass_kernel_spmd(_NC_CACHE, in_maps,
                               core_ids=list(range(NCORES)))
    outs = []
    for core in range(NCORES):
        qa = res.results[core]["qa"].reshape(A6, N_DESC, CH, L_MAX)
        braw = res.results[core]["braw"].reshape(A6, 2, N_DESC, HB)
        B = np.concatenate([braw[:, 0], braw[:, 1]], axis=-1)  # [A6, 8, CH]
        q = qa - 0.5 * B[..., None]                  # [a, d, c, l]
        outs.append(q.transpose(2, 0, 1, 3).reshape(CA, N_DESC, L_MAX))
    q = np.concatenate(outs, 0)[:N_ATOMS]
    return np.ascontiguousarray(q.astype(np.float32))


if __name__ == "__main__":
    if os.path.exists("/tmp/ref_cache.npz"):
        z = np.load("/tmp/ref_cache.npz")
        inputs = {k: z[k] for k in
                  ("types", "positions", "angular_neighbors", "c_table")}
        exp = z["exp"]
    else:
        import reference
        inputs = {k: np.asarray(v) for k, v in reference.setup_inputs().items()}
        exp = np.asarray(reference.reference(**inputs))
    act = kernel(**inputs)
    rel = np.linalg.norm(act - exp) / np.linalg.norm(exp)
    print("Relative error:", rel)
